# revision 1
# baseline (speedup 1.0000x reference)
"""Bass/Trainium2 kernel for nn_BaseAttention (B=2, S=2048, H=1024, NH=16, HD=64).

Sharding: 8 cores = 2 batches x 4 head-groups (4 heads each core).
Each core computes, for its (batch b, head-group hb):
    qkv slice -> attention over masked keys -> partial out-projection
and writes partial^T [H, S].  Host sums the 4 partials per batch and
transposes.

Key algorithmic choices:
  * Masked keys are packed out on the host (attention only runs over the
    ~50% surviving keys, padded to a multiple of 128).  Padding slots get a
    -30000 logit bias so exp() underflows to 0.
  * Scores are computed in S^T layout [k_part, q_free]; the mask bias is a
    per-partition ACT bias fused into the exp() activation together with the
    1/sqrt(HD) scale:  P^T = exp(scale*S^T + bias).
  * Softmax denominator comes free as a 65th "ones" column of V in the
    P^T @ V_aug matmul; the division commutes to a per-head scalar multiply
    after the AV matmul.
  * No row-max subtraction (logits are provably small for this problem:
    |logit| < ~4, exp() cannot overflow in fp32).
  * Matmuls run as float32r (TF32-like, 1 cyc/row) or float32 (4 cyc/row)
    per-stage, configurable below.

Measured (8 cores, hidden/mask from jax.random.key(0), KP=1152):
  rel err vs fp32 jax reference: 2.8e-4 (all-f32r; absmax err 3.8e-5 on
  absmax 0.135).  TimelineSim NEFF time: ~163 us/core.  Fallbacks:
  av+out in f32 -> 306 us @ ~1.5e-4; all-f32 -> 439 us @ ~1e-6.
"""

import numpy as np

import concourse.bass as bass
import concourse.mybir as mybir
import concourse.tile as tile
from concourse import bacc
from concourse import bass_utils

B, S, H = 2, 2048, 1024
NH, HD = 16, 64
SCALE = HD ** -0.5
NCORES = 8
CPB = NCORES // B          # cores per batch = 4
NHL = NH // CPB            # local heads per core = 4
QD = NHL * HD              # local head-dim total = 256

F32 = mybir.dt.float32
F32R = mybir.dt.float32r

# per-stage matmul dtype: "f32r" (fast, ~1.3e-4 rel) or "f32" (exact, 4x slower)
STAGE_DT = {
    "qk": "f32r",      # Q/K projections (feeds softmax: error-insensitive)
    "v": "f32r",       # V projection
    "scores": "f32r",  # Q.K^T
    "av": "f32r",      # P.V
    "out": "f32r",     # out-projection
}


def _chunks(total, size):
    out = []
    o = 0
    while o < total:
        c = min(size, total - o)
        out.append((o, c))
        o += c
    return out


def build_kernel(KP, S_=S, Hd=H, NHL_=NHL, stage_dt=None, phases=("proj", "attn", "out")):
    """Build the per-core Bass program.  All cores run this same NEFF."""
    stage_dt = dict(STAGE_DT, **(stage_dt or {}))
    assert stage_dt["qk"] == stage_dt["v"], "xpT feeds both K and V projections"

    def mdt(stage):
        return F32R if stage_dt[stage] == "f32r" else F32

    QD_ = NHL_ * HD
    HT = Hd // 128          # k-tiles over hidden dim
    MT = QD_ // 128         # partition-tiles over local q/k/v dims
    KT = KP // 128          # partition-tiles over packed keys
    PW = min(1024, S_)      # proj psum width
    SC = _chunks(S_, 512)   # rhs chunks over S
    KC = _chunks(KP, 512)   # rhs chunks over packed keys

    nc = bacc.Bacc("TRN2")
    xT = nc.dram_tensor("xT", [Hd, S_], mdt("qk"), kind="ExternalInput")
    xpT = nc.dram_tensor("xpT", [Hd, KP], mdt("qk"), kind="ExternalInput")
    wqT = nc.dram_tensor("wqT", [Hd, QD_], mdt("qk"), kind="ExternalInput")
    wkT = nc.dram_tensor("wkT", [Hd, QD_], mdt("qk"), kind="ExternalInput")
    wvT = nc.dram_tensor("wvT", [Hd, QD_], mdt("qk"), kind="ExternalInput")
    woT = nc.dram_tensor("woT", [QD_, Hd], mdt("out"), kind="ExternalInput")
    bk = nc.dram_tensor("bk", [128, KT], F32, kind="ExternalInput")
    outT = nc.dram_tensor("outT", [Hd, S_], F32, kind="ExternalOutput")

    with tile.TileContext(nc) as tc:
        with tile.TileContext.tile_pool(tc, name="wts", bufs=1) as wp, \
             tile.TileContext.tile_pool(tc, name="proj", bufs=1) as jp:
            # ---- persistent weights / proj outputs
            wq_sb = wp.tile([128, HT, QD_], mdt("qk"))
            wk_sb = wp.tile([128, HT, QD_], mdt("qk"))
            wv_sb = wp.tile([128, HT, QD_], mdt("qk"))
            wo_sb = wp.tile([128, MT, Hd], mdt("out"))
            bk_sb = wp.tile([128, KT], F32)
            nc.sync.dma_start(out=bk_sb, in_=bk.ap())

            qT_sb = jp.tile([128, MT, S_], mdt("scores"))     # Q^T (head dims on partitions)
            kT_sb = jp.tile([128, MT, KP], mdt("scores"))     # K^T over packed keys
            va_sb = jp.tile([128, KT, NHL_, HD + 1], mdt("av"))  # V rows + ones col
            aT_sb = jp.tile([128, MT, S_], mdt("out"))     # attention out^T (normalized)
            nc.vector.memset(va_sb[:, :, :, HD:HD + 1].bitcast(F32), 1.0)

            with tile.TileContext.tile_pool(tc, name="xp", bufs=1) as xp:
                xT_sb = xp.tile([128, HT, S_], mdt("qk"))
                xpT_sb = xp.tile([128, HT, KP], mdt("qk"))
                # K/V-path data first: attention's critical path starts with
                # K^T and V, so their DMAs and projections lead.
                for ht in range(HT):
                    nc.sync.dma_start(out=xpT_sb[:, ht, :],
                                      in_=xpT.ap()[ht * 128:(ht + 1) * 128, :])
                    nc.scalar.dma_start(out=wk_sb[:, ht, :],
                                        in_=wkT.ap()[ht * 128:(ht + 1) * 128, :])
                    nc.scalar.dma_start(out=wv_sb[:, ht, :],
                                        in_=wvT.ap()[ht * 128:(ht + 1) * 128, :])
                for ht in range(HT):
                    nc.sync.dma_start(out=xT_sb[:, ht, :],
                                      in_=xT.ap()[ht * 128:(ht + 1) * 128, :])
                    nc.scalar.dma_start(out=wq_sb[:, ht, :],
                                        in_=wqT.ap()[ht * 128:(ht + 1) * 128, :])
                for mt in range(MT):
                    nc.scalar.dma_start(out=wo_sb[:, mt, :],
                                        in_=woT.ap()[mt * 128:(mt + 1) * 128, :])

                with tile.TileContext.tile_pool(tc, name="pj", bufs=2,
                                                space="PSUM") as pjp, \
                     tile.TileContext.tile_pool(tc, name="pv", bufs=2,
                                                space="PSUM") as pvp:
                    # K^T projection (output-transposed orientation)
                    for mt in range(MT if "proj" in phases else 0):
                        for po, pw in _chunks(KP, PW):
                            ps = pjp.tile([128, PW], F32, tag="pj",
                                          name="ps_proj")
                            for kt in range(HT):
                                for co, cw in _chunks(pw, 512):
                                    nc.tensor.matmul(
                                        ps[:, co:co + cw],
                                        wk_sb[:, kt, mt * 128:(mt + 1) * 128],
                                        xpT_sb[:, kt, po + co:po + co + cw],
                                        start=(kt == 0), stop=(kt == HT - 1))
                            nc.scalar.copy(kT_sb[:, mt, po:po + pw], ps[:, 0:pw])
                    # V projection (natural orientation: keys on partitions)
                    for st in range(KT if "proj" in phases else 0):
                        psv = pvp.tile([128, QD_], F32, tag="pv", name="ps_v")
                        for kt in range(HT):
                            nc.tensor.matmul(
                                psv,
                                xpT_sb[:, kt, st * 128:(st + 1) * 128],
                                wv_sb[:, kt, :],
                                start=(kt == 0), stop=(kt == HT - 1))
                        nc.vector.tensor_copy(
                            va_sb[:, st, :, 0:HD],
                            psv.rearrange("p (h d) -> p h d", h=NHL_))

                # Q^T projection, kt-outer: all four output chunks accumulate
                # in parallel so the last xT tile's arrival gates only ~2us.
                with tile.TileContext.tile_pool(tc, name="pq", bufs=1,
                                                space="PSUM") as pqp:
                    if "proj" in phases:
                        qchunks = [(mt, po, pw)
                                   for mt in range(MT)
                                   for po, pw in _chunks(S_, PW)]
                        pss_q = [pqp.tile([128, PW], F32, tag=f"pq{i}",
                                          name=f"ps_q{i}")
                                 for i in range(len(qchunks))]
                        for kt in range(HT):
                            for i, (mt, po, pw) in enumerate(qchunks):
                                for co, cw in _chunks(pw, 512):
                                    nc.tensor.matmul(
                                        pss_q[i][:, co:co + cw],
                                        wq_sb[:, kt, mt * 128:(mt + 1) * 128],
                                        xT_sb[:, kt, po + co:po + co + cw],
                                        start=(kt == 0), stop=(kt == HT - 1))
                        for i, (mt, po, pw) in enumerate(qchunks):
                            if i % 2 == 0:
                                nc.scalar.copy(qT_sb[:, mt, po:po + pw],
                                               pss_q[i][:, 0:pw])
                            else:
                                nc.vector.tensor_copy(qT_sb[:, mt, po:po + pw],
                                                      pss_q[i][:, 0:pw])

            # ---- attention, head by head
            with tile.TileContext.tile_pool(tc, name="ps", bufs=3, space="PSUM") as psp, \
                 tile.TileContext.tile_pool(tc, name="po", bufs=1, space="PSUM") as pop, \
                 tile.TileContext.tile_pool(tc, name="pp", bufs=4) as ppp, \
                 tile.TileContext.tile_pool(tc, name="dv", bufs=2) as dvp:
                for h in range(NHL_ if "attn" in phases else 0):
                    mtq = (h * HD) // 128
                    rb = (h * HD) % 128
                    for po_, pw in _chunks(S_, 1024):
                        pso = pop.tile([HD + 1, min(1024, S_)], F32, tag="po",
                                       name="ps_o")
                        for kt in range(KT):
                            pss = psp.tile([128, min(1024, S_)], F32, tag="ps",
                                           name="ps_s")
                            for co, cw in _chunks(pw, 512):
                                q0 = po_ + co
                                nc.tensor.matmul(
                                    pss[:, co:co + cw],
                                    kT_sb[rb:rb + HD, mtq,
                                               kt * 128:(kt + 1) * 128],
                                    qT_sb[rb:rb + HD, mtq, q0:q0 + cw],
                                    start=True, stop=True)
                            pex = ppp.tile([128, min(1024, S_)], mdt("av"),
                                           tag="pex", name="p_exp")
                            nc.scalar.activation(
                                out=pex[:, 0:pw], in_=pss[:, 0:pw],
                                func=mybir.ActivationFunctionType.Exp,
                                bias=bk_sb[:, kt:kt + 1], scale=SCALE)
                            for co, cw in _chunks(pw, 512):
                                nc.tensor.matmul(
                                    pso[:, co:co + cw],
                                    va_sb[:, kt, h, :],
                                    pex[:, co:co + cw],
                                    start=(kt == 0), stop=(kt == KT - 1))
                        # evacuate AV psum immediately (frees pso), then
                        # normalize off the critical path from the SBUF copy
                        onum = dvp.tile([HD + 1, min(1024, S_)], F32,
                                        tag="onum", name="onum")
                        nc.vector.tensor_copy(onum, pso)
                        recip = dvp.tile([1, min(1024, S_)], F32, tag="recip",
                                         name="recip")
                        nc.vector.reciprocal(recip, onum[HD:HD + 1, :])
                        bc = dvp.tile([HD, min(1024, S_)], F32, tag="bc",
                                      name="bc")
                        nc.gpsimd.partition_broadcast(bc, recip)
                        nc.vector.tensor_mul(
                            aT_sb[rb:rb + HD, mtq, po_:po_ + pw],
                            onum[0:HD, 0:pw], bc[:, 0:pw])

            # ---- out-projection: partial^T[j, q] = W_o^T-slice . A^T
            with tile.TileContext.tile_pool(tc, name="pf", bufs=4, space="PSUM") as pfp, \
                 tile.TileContext.tile_pool(tc, name="so", bufs=4) as sop:
                dmaengs = [nc.sync, nc.scalar]
                di = 0
                for jt in range(HT if "out" in phases else 0):
                    for ho, hwid in _chunks(S_, 1024):
                        psf = pfp.tile([128, min(1024, S_)], F32, tag="pf",
                                       name="ps_f")
                        for kt in range(MT):
                            for co, cw in _chunks(hwid, 512):
                                nc.tensor.matmul(
                                    psf[:, co:co + cw],
                                    wo_sb[:, kt, jt * 128:(jt + 1) * 128],
                                    aT_sb[:, kt, ho + co:ho + co + cw],
                                    start=(kt == 0), stop=(kt == MT - 1))
                        stg = sop.tile([128, min(1024, S_)], F32, tag="stg",
                                       name="stage")
                        if (jt + ho) % 2 == 0:
                            nc.scalar.copy(stg, psf[:, 0:hwid])
                        else:
                            nc.vector.tensor_copy(stg, psf[:, 0:hwid])
                        dmaengs[di % 2].dma_start(
                            out=outT.ap()[jt * 128:(jt + 1) * 128, ho:ho + hwid],
                            in_=stg)
                        di += 1

    nc.compile()
    return nc


def _prep_inputs(hidden_states, attention_mask, w_qkv, w_out):
    """Shard + transpose inputs for the 8 cores.  Returns (KP, in_maps)."""
    hs = np.asarray(hidden_states, dtype=np.float32)
    mask = np.asarray(attention_mask)
    wqkv = np.asarray(w_qkv, dtype=np.float32)
    wo = np.asarray(w_out, dtype=np.float32)

    idxs = [np.nonzero(mask[b] != 0)[0] for b in range(B)]
    counts = [len(ix) for ix in idxs]
    KP = max(128, ((max(counts) + 127) // 128) * 128)

    xTs, xpTs, biases = [], [], []
    for b in range(B):
        xTs.append(np.ascontiguousarray(hs[b].T))
        xp = np.zeros((KP, H), dtype=np.float32)
        xp[:counts[b]] = hs[b][idxs[b]]
        xpTs.append(np.ascontiguousarray(xp.T))
        bias = np.zeros(KP, dtype=np.float32)
        bias[counts[b]:] = -30000.0
        biases.append(np.ascontiguousarray(bias.reshape(KP // 128, 128).T))

    in_maps = []
    for c in range(NCORES):
        b, hb = c // CPB, c % CPB
        sl = slice(hb * QD, (hb + 1) * QD)
        in_maps.append({
            "xT": xTs[b],
            "xpT": xpTs[b],
            "wqT": np.ascontiguousarray(wqkv[sl, :].T),
            "wkT": np.ascontiguousarray(wqkv[H + sl.start:H + sl.stop, :].T),
            "wvT": np.ascontiguousarray(wqkv[2 * H + sl.start:2 * H + sl.stop, :].T),
            "woT": np.ascontiguousarray(wo[:, sl].T),
            "bk": biases[b],
        })
    return KP, in_maps


_NC_CACHE = {}


def kernel(hidden_states, attention_mask, w_qkv, w_out):
    KP, in_maps = _prep_inputs(hidden_states, attention_mask, w_qkv, w_out)
    key = (KP, tuple(sorted(STAGE_DT.items())))
    if key not in _NC_CACHE:
        _NC_CACHE[key] = build_kernel(KP)
    nc = _NC_CACHE[key]
    res = bass_utils.run_bass_kernel_spmd(nc, in_maps,
                                          core_ids=list(range(NCORES)))
    out = np.empty((B, S, H), dtype=np.float32)
    for b in range(B):
        acc = res.results[b * CPB]["outT"].astype(np.float32).copy()
        for c in range(b * CPB + 1, (b + 1) * CPB):
            acc += res.results[c]["outT"]
        out[b] = acc.T
    return out



# revision 4
# speedup vs baseline: 1.1163x; 1.1163x over previous
"""Bass/Trainium2 kernel for nn_BaseAttention (B=2, S=2048, H=1024, NH=16, HD=64).

Sharding: 8 cores = 2 batches x 4 head-groups (4 heads each core).
Each core computes, for its (batch b, head-group g):
    qkv projections -> masked attention -> partial out-projection^T [H, S].
Host sums the 4 partials per batch and transposes.

Key choices (driven by the TimelineSim cost model: matmul cost =
out_free_size x cycles_per_row, fp16=1 cyc, fp8+DoubleRow=0.5 cyc;
activation = free_size x 0.833ns):
  * Masked keys packed on host (KP = padded surviving keys); pad keys get a
    -30000 exp bias.
  * Q/K/V projections run as fp8e4 DoubleRow matmuls on a hi/lo split of
    X^T and W^T (3-term compensation: hi*hi + hi*lo + lo*hi), giving
    ~tf32-grade accuracy at 0.75x the fp16 matmul cost.  W is prescaled by
    WS=32 to avoid fp8 denormals; the 1/WS^2 folds into the exp scale and
    the 1/WS for V folds into the V-evac copy.
  * Scores S^T = K^T.T @ Q^T in fp16 ([keys, queries] psum tiles of
    [128,1024]); exp fused with mask bias + scale on ACT -> fp16 pex.
  * AV in natural orientation out[q,65] (ones column -> denominator),
    normalized per-partition via reciprocal + tensor_scalar_mul,
    then PE-transposed (identity matmul) to A^T for the out-projection.
  * Out-projection fp16, partial^T [1024, 2048] -> fp16 DMA out.
Emission order pipelines: scores(h) interleaves with AV(h-1) so the ACT
exp stream (the ~75us bottleneck) never stalls.
"""

import numpy as np
import ml_dtypes

import concourse.bass as bass
import concourse.mybir as mybir
import concourse.tile as tile
from concourse import bacc
from concourse import bass_utils

B, S, H = 2, 2048, 1024
NH, HD = 16, 64
SCALE = HD ** -0.5
NCORES = 8
CPB = NCORES // B          # cores per batch = 4
NHL = NH // CPB            # local heads per core = 4
QD = NHL * HD              # local q/k/v dims = 256
WS = 32.0                  # fp8 weight prescale (avoids e4m3 denormals)

HT = H // 128              # hidden-dim 128-tiles = 8
NTP = HT // 2              # hidden-tile pairs = 4
QBW = 256                  # xq query-block width
NQB = S // QBW             # 8

F32 = mybir.dt.float32
F16 = mybir.dt.float16
F8 = mybir.dt.float8e4
DRM = mybir.MatmulPerfMode.DoubleRow
E4 = ml_dtypes.float8_e4m3
EXP = mybir.ActivationFunctionType.Exp


def build_kernel(KP):
    KT = KP // 128
    nc = bacc.Bacc("TRN2")
    xq = nc.dram_tensor("xq", [128, NQB, HT, 2, QBW], F8, kind="ExternalInput")
    xp = nc.dram_tensor("xp", [128, KT, HT, 2, 128], F8, kind="ExternalInput")
    wqx = nc.dram_tensor("wqx", [128, HT, 2, QD], F8, kind="ExternalInput")
    wkx = nc.dram_tensor("wkx", [128, HT, 2, QD], F8, kind="ExternalInput")
    wvx = nc.dram_tensor("wvx", [128, HT, 2, QD], F8, kind="ExternalInput")
    woT = nc.dram_tensor("woT", [QD, H], F16, kind="ExternalInput")
    bk = nc.dram_tensor("bk", [128, KT], F32, kind="ExternalInput")
    ident = nc.dram_tensor("ident", [128, 128], F16, kind="ExternalInput")
    outT = nc.dram_tensor("outT", [H, S], F16, kind="ExternalOutput")

    with tile.TileContext(nc) as tc:
        with tile.TileContext.tile_pool(tc, name="wts", bufs=1) as wp:
            wq_sb = wp.tile([128, HT, 2, QD], F8)
            wk_sb = wp.tile([128, HT, 2, QD], F8)
            wv_sb = wp.tile([128, HT, 2, QD], F8)
            wo_sb = wp.tile([128, 2, H], F16)
            bk_sb = wp.tile([128, KT], F32)
            id_sb = wp.tile([128, 128], F16)
            qT_sb = wp.tile([128, 2, S], F16)       # Q^T (q-dims on partitions)
            kT_sb = wp.tile([128, 2, KP], F16)      # K^T over packed keys
            va_sb = wp.tile([128, KT, NHL, HD + 1], F16)  # V rows + ones col
            a_sb = wp.tile([128, S // 128, QD], F16)      # A natural [q, a]
            aT_sb = wp.tile([128, 2, S], F16)             # A^T [a, q]

            nc.sync.dma_start(out=bk_sb, in_=bk.ap())
            nc.sync.dma_start(out=id_sb, in_=ident.ap())
            nc.vector.memset(va_sb[:, :, :, HD:HD + 1], 1.0)
            nc.sync.dma_start(out=wk_sb, in_=wkx.ap())
            nc.sync.dma_start(out=wq_sb, in_=wqx.ap())
            nc.sync.dma_start(out=wv_sb, in_=wvx.ap())
            for mt in range(2):
                nc.sync.dma_start(out=wo_sb[:, mt, :],
                                  in_=woT.ap()[mt * 128:(mt + 1) * 128, :])

            # ------------- projections (fp8 hi/lo DoubleRow) -------------
            with tile.TileContext.tile_pool(tc, name="xin", bufs=1) as xpool:
                xq_sb = xpool.tile([128, NQB, HT, 2, QBW], F8)
                xp_sb = xpool.tile([128, KT, HT, 2, 128], F8)
                # DMA order: first key-block, then all query blocks (Q-proj
                # path), then remaining key blocks.
                nc.sync.dma_start(out=xp_sb[:, 0], in_=xp.ap()[:, 0])
                for qb in range(NQB):
                    nc.sync.dma_start(out=xq_sb[:, qb], in_=xq.ap()[:, qb])
                for kb in range(1, KT):
                    nc.sync.dma_start(out=xp_sb[:, kb], in_=xp.ap()[:, kb])

                def hilo(ps, lhs_main, rhs_main, lhs_cross, rhs_cross):
                    """12 DR matmuls: 4 hi*hi pair-steps + 8 cross steps."""
                    for t in range(NTP):
                        nc.tensor.matmul(ps, lhs_main(t), rhs_main(t),
                                         start=(t == 0), stop=False,
                                         perf_mode=DRM)
                    for ht in range(HT):
                        nc.tensor.matmul(ps, lhs_cross(ht), rhs_cross(ht),
                                         start=False, stop=(ht == HT - 1),
                                         perf_mode=DRM)

                with tile.TileContext.tile_pool(tc, name="pk", bufs=2,
                                                space="PSUM") as pkp, \
                     tile.TileContext.tile_pool(tc, name="pv", bufs=2,
                                                space="PSUM") as pvp, \
                     tile.TileContext.tile_pool(tc, name="pq", bufs=2,
                                                space="PSUM") as pqp:

                    def kproj(cb, kb):
                        ps = pkp.tile([128, 128], F32, tag="pk", name="ps_k")
                        cs = slice(cb * 128, (cb + 1) * 128)
                        hilo(ps,
                             lambda t: wk_sb[:, 2 * t:2 * t + 2, 1, cs],
                             lambda t: xp_sb[:, kb, 2 * t:2 * t + 2, 0, :],
                             lambda ht: wk_sb[:, ht, :, cs],
                             lambda ht: xp_sb[:, kb, ht, :, :])
                        nc.vector.tensor_copy(
                            kT_sb[:, cb, kb * 128:(kb + 1) * 128], ps)

                    def vproj(kb):
                        ps = pvp.tile([128, QD], F32, tag="pv", name="ps_v")
                        hilo(ps,
                             lambda t: xp_sb[:, kb, 2 * t:2 * t + 2, 0, :],
                             lambda t: wv_sb[:, 2 * t:2 * t + 2, 1, :],
                             lambda ht: xp_sb[:, kb, ht, :, :],
                             lambda ht: wv_sb[:, ht, :, :])
                        # V carries a WS factor; remove it at evac time.
                        nc.vector.tensor_scalar_mul(
                            va_sb[:, kb, :, 0:HD],
                            ps.rearrange("p (h d) -> p h d", h=NHL), 1.0 / WS)

                    def qproj(cb, qb):
                        ps = pqp.tile([128, QBW], F32, tag="pq", name="ps_q")
                        cs = slice(cb * 128, (cb + 1) * 128)
                        hilo(ps,
                             lambda t: wq_sb[:, 2 * t:2 * t + 2, 1, cs],
                             lambda t: xq_sb[:, qb, 2 * t:2 * t + 2, 0, :],
                             lambda ht: wq_sb[:, ht, :, cs],
                             lambda ht: xq_sb[:, qb, ht, :, :])
                        nc.vector.tensor_copy(
                            qT_sb[:, cb, qb * QBW:(qb + 1) * QBW], ps)

                    kproj(0, 0)
                    kproj(1, 0)
                    vproj(0)
                    for qb in range(NQB):
                        qproj(0, qb)
                        qproj(1, qb)
                    for kb in range(1, KT):
                        kproj(0, kb)
                        kproj(1, kb)
                        vproj(kb)

            # ------------- attention + out-projection -------------
            with tile.TileContext.tile_pool(tc, name="pex", bufs=2) as pexp, \
                 tile.TileContext.tile_pool(tc, name="sc", bufs=2,
                                            space="PSUM") as scp, \
                 tile.TileContext.tile_pool(tc, name="av", bufs=1,
                                            space="PSUM") as avp, \
                 tile.TileContext.tile_pool(tc, name="tp", bufs=1,
                                            space="PSUM") as tpp, \
                 tile.TileContext.tile_pool(tc, name="op", bufs=2,
                                            space="PSUM") as opp:

                pex_tiles = {}

                def new_pex(h):
                    pex_tiles[h] = pexp.tile([128, KT, S], F16, tag="pex",
                                             name=f"pex{h}")

                def scores_kt(h, kt):
                    pex_h = pex_tiles[h]
                    mtq, rb = h // 2, 64 * (h % 2)
                    for half in range(2):
                        ps = scp.tile([128, 1024], F32, tag="sc", name="ps_s")
                        q0 = half * 1024
                        for co in (0, 512):
                            nc.tensor.matmul(
                                ps[:, co:co + 512],
                                kT_sb[rb:rb + 64, mtq, kt * 128:(kt + 1) * 128],
                                qT_sb[rb:rb + 64, mtq, q0 + co:q0 + co + 512],
                                start=True, stop=True)
                        nc.scalar.activation(
                            out=pex_h[:, kt, q0:q0 + 1024], in_=ps,
                            func=EXP, bias=bk_sb[:, kt:kt + 1],
                            scale=SCALE / (WS * WS))

                def av_block(h, qts):
                    pex_h = pex_tiles[h]
                    for qt in qts:
                        pa = avp.tile([128, HD + 1], F32, tag="av", name="pa")
                        for kt in range(KT):
                            nc.tensor.matmul(
                                pa, pex_h[:, kt, qt * 128:(qt + 1) * 128],
                                va_sb[:, kt, h, :],
                                start=(kt == 0), stop=(kt == KT - 1))
                        rcp = wp.tile([128, 1], F32, tag="rcp", bufs=4,
                                      name="rcp")
                        nc.vector.reciprocal(rcp, pa[:, HD:HD + 1])
                        nc.vector.tensor_scalar_mul(
                            a_sb[:, qt, h * HD:(h + 1) * HD], pa[:, 0:HD], rcp)

                def transpose_block(at, qg):
                    tp_t = tpp.tile([128, 512], F16, tag="tp", name="tp")
                    for j in range(4):
                        qt = qg * 4 + j
                        nc.tensor.transpose(
                            tp_t[:, j * 128:(j + 1) * 128],
                            a_sb[:, qt, at * 128:(at + 1) * 128], id_sb)
                    nc.vector.tensor_copy(
                        aT_sb[:, at, qg * 512:(qg + 1) * 512], tp_t)

                def outproj(qq):
                    for jt in range(HT):
                        po = opp.tile([128, 512], F32, tag="op", name="po")
                        for mt in range(2):
                            nc.tensor.matmul(
                                po, wo_sb[:, mt, jt * 128:(jt + 1) * 128],
                                aT_sb[:, mt, qq * 512:(qq + 1) * 512],
                                start=(mt == 0), stop=(mt == 1))
                        stg = wp.tile([128, 512], F16, tag="stg", bufs=4,
                                      name="stg")
                        # ACT is idle in the tail (exp stream done): split the
                        # psum evacuations between DVE and ACT.
                        if jt % 2 == 0:
                            nc.vector.tensor_copy(stg, po)
                        else:
                            nc.scalar.copy(stg, po)
                        nc.sync.dma_start(
                            out=outT.ap()[jt * 128:(jt + 1) * 128,
                                          qq * 512:(qq + 1) * 512],
                            in_=stg)

                new_pex(0)
                for kt in range(KT):
                    scores_kt(0, kt)
                for h in (1, 2, 3):
                    new_pex(h)
                    for kt in range(KT):
                        scores_kt(h, kt)
                        qts = [q for q in (2 * kt, 2 * kt + 1) if q < 16]
                        av_block(h - 1, qts)
                        # at0 (heads 0,1) transposes as soon as h1 rows done
                        if h == 2 and kt % 2 == 1:
                            transpose_block(0, kt // 2)
                # tail: last head's AV, at1 transposes, out-projection
                for qg in range(4):
                    av_block(3, range(4 * qg, 4 * qg + 4))
                    transpose_block(1, qg)
                    outproj(qg)

    nc.compile()
    return nc


def _hilo8(a):
    hi = np.asarray(a, np.float32).astype(E4)
    lo = (np.asarray(a, np.float32) - hi.astype(np.float32)).astype(E4)
    return hi, lo


def _pack_x(xT, blk):
    """[1024, N] -> [128, N//blk, 8, 2, blk] with (hi, lo) on axis 3."""
    n = xT.shape[1]
    hi, lo = _hilo8(xT)
    out = np.stack([hi.reshape(HT, 128, n // blk, blk),
                    lo.reshape(HT, 128, n // blk, blk)], axis=3)
    return np.ascontiguousarray(out.transpose(1, 2, 0, 3, 4))


def _pack_w(wT):
    """[1024, QD] -> [128, 8, 2, QD] with (lo, hi) on axis 2."""
    hi, lo = _hilo8(wT * WS)
    out = np.stack([lo.reshape(HT, 128, QD), hi.reshape(HT, 128, QD)], axis=2)
    return np.ascontiguousarray(out.transpose(1, 0, 2, 3))


def _prep_inputs(hidden_states, attention_mask, w_qkv, w_out):
    """Shard + pack inputs for the 8 cores.  Returns (KP, in_maps)."""
    hs = np.asarray(hidden_states, dtype=np.float32)
    mask = np.asarray(attention_mask)
    wqkv = np.asarray(w_qkv, dtype=np.float32)
    wo = np.asarray(w_out, dtype=np.float32)

    idxs = [np.nonzero(mask[b] != 0)[0] for b in range(B)]
    counts = [len(ix) for ix in idxs]
    KP = max(128, ((max(counts) + 127) // 128) * 128)
    KT = KP // 128

    xqs, xps, bks = [], [], []
    for b in range(B):
        xqs.append(_pack_x(np.ascontiguousarray(hs[b].T), QBW))
        xpad = np.zeros((KP, H), dtype=np.float32)
        xpad[:counts[b]] = hs[b][idxs[b]]
        xps.append(_pack_x(np.ascontiguousarray(xpad.T), 128))
        bias = np.zeros((KT, 128), dtype=np.float32)
        bias.reshape(-1)[counts[b]:] = -30000.0
        bks.append(np.ascontiguousarray(bias.T))

    ident = np.eye(128, dtype=np.float16)
    in_maps = []
    for c in range(NCORES):
        b, g = c // CPB, c % CPB
        sl = slice(g * QD, (g + 1) * QD)
        in_maps.append({
            "xq": xqs[b],
            "xp": xps[b],
            "wqx": _pack_w(np.ascontiguousarray(wqkv[sl, :].T)),
            "wkx": _pack_w(np.ascontiguousarray(
                wqkv[H + sl.start:H + sl.stop, :].T)),
            "wvx": _pack_w(np.ascontiguousarray(
                wqkv[2 * H + sl.start:2 * H + sl.stop, :].T)),
            "woT": np.ascontiguousarray(wo[:, sl].T.astype(np.float16)),
            "bk": bks[b],
            "ident": ident,
        })
    return KP, in_maps


_NC_CACHE = {}


def kernel(hidden_states, attention_mask, w_qkv, w_out):
    KP, in_maps = _prep_inputs(hidden_states, attention_mask, w_qkv, w_out)
    if KP not in _NC_CACHE:
        _NC_CACHE[KP] = build_kernel(KP)
    nc = _NC_CACHE[KP]
    res = bass_utils.run_bass_kernel_spmd(nc, in_maps,
                                          core_ids=list(range(NCORES)))
    out = np.empty((B, S, H), dtype=np.float32)
    for b in range(B):
        acc = res.results[b * CPB]["outT"].astype(np.float32)
        for c in range(b * CPB + 1, (b + 1) * CPB):
            acc = acc + res.results[c]["outT"].astype(np.float32)
        out[b] = acc.T
    return out


# revision 27
# speedup vs baseline: 1.4430x; 1.2927x over previous
"""Bass/Trainium2 kernel for nn_BaseAttention (B=2, S=2048, H=1024, NH=16, HD=64).

Sharding: 8 cores = 2 batches x 4 head-groups (4 heads each core).
Each core computes, for its (batch b, head-group g):
    qkv projections -> masked attention -> partial out-projection^T [H, S].
Host sums the 4 partials per batch and transposes.

Cost-model-driven choices (TimelineSim: matmul = out_free x cyc/row with
fp16=1, fp8+DoubleRow=0.5; activation = free x 0.833ns + ~185ns/instr):
  * Masked keys packed on host (KP = padded surviving keys); pad keys get a
    -30000 exp bias.
  * Q/K/V projections: fp8e4 DoubleRow on hi/lo splits of X^T and W^T
    (3-term compensation hi*hi + hi*lo + lo*hi) -> tf32-grade accuracy at
    0.75x fp16 matmul cost.  W prescaled by WS=32 (dodges fp8 denormals);
    1/WS^2 folds into the exp scale, 1/WS into the V evac.
  * Scores S^T in fp16, psum [128,1024] tiles; exp on ACT (the ~75us
    bottleneck) fused with mask bias + scale -> fp16 pex.
  * AV natural orientation out[q, 65] (ones column = softmax denominator),
    per-qt normalize (reciprocal + tensor_scalar_mul) -> A, then SBUF->SBUF
    DMA-transpose (XBAR) to A^T.  No PE/psum cost for the transpose.
  * Out-projection fp16 -> partial^T [1024, 2048] fp16 DMA out.
Emission order = per-engine execution order.  The exp stream is split into
a half0 pass (queries 0-1023, all heads) then a half1 pass, so it starts
after only ~1/2 the input DMA, and the out-projection for queries 0-1023
hides inside the half1 exp window.  Only the last head's half1 AV + the
last out-projection half remain as a tail.
"""

import numpy as np
import ml_dtypes

import concourse.bass as bass
import concourse.mybir as mybir
import concourse.tile as tile
from concourse import bacc
from concourse import bass_utils

B, S, H = 2, 2048, 1024
NH, HD = 16, 64
SCALE = HD ** -0.5
NCORES = 8
CPB = NCORES // B          # cores per batch = 4
NHL = NH // CPB            # local heads per core = 4
QD = NHL * HD              # local q/k/v dims = 256
WS = 32.0                  # fp8 weight prescale (avoids e4m3 denormals)

HT = H // 128              # hidden-dim 128-tiles = 8
NTP = HT // 2              # hidden-tile pairs = 4
QBW = 256                  # xq query-block width
NQB = S // QBW             # 8

F32 = mybir.dt.float32
F16 = mybir.dt.float16
F8 = mybir.dt.float8e4
DRM = mybir.MatmulPerfMode.DoubleRow
E4 = ml_dtypes.float8_e4m3
EXP = mybir.ActivationFunctionType.Exp


def build_kernel(KP, phases=("av", "tr", "op")):
    KT = KP // 128
    nc = bacc.Bacc("TRN2")
    xq = nc.dram_tensor("xq", [128, NQB, HT, 2, QBW], F8, kind="ExternalInput")
    xp = nc.dram_tensor("xp", [128, KT, HT, 2, 128], F8, kind="ExternalInput")
    wqx = nc.dram_tensor("wqx", [128, HT, 2, QD], F8, kind="ExternalInput")
    wkx = nc.dram_tensor("wkx", [128, HT, 2, QD], F8, kind="ExternalInput")
    wvx = nc.dram_tensor("wvx", [128, HT, 2, QD], F8, kind="ExternalInput")
    woT = nc.dram_tensor("woT", [QD, H], F16, kind="ExternalInput")
    bk = nc.dram_tensor("bk", [128, KT], F32, kind="ExternalInput")
    ident = nc.dram_tensor("ident", [128, 128], F16, kind="ExternalInput")
    outT = nc.dram_tensor("outT", [H, S], F16, kind="ExternalOutput")

    do_av = "av" in phases
    do_tr = "tr" in phases
    do_op = "op" in phases

    with tile.TileContext(nc) as tc:
        with tile.TileContext.tile_pool(tc, name="wts", bufs=1) as wp, \
             tile.TileContext.tile_pool(tc, name="pex", bufs=3) as pexp, \
             tile.TileContext.tile_pool(tc, name="sc", bufs=2,
                                        space="PSUM") as scp, \
             tile.TileContext.tile_pool(tc, name="av", bufs=2,
                                        space="PSUM") as avp, \
             tile.TileContext.tile_pool(tc, name="ps", bufs=2,
                                        space="PSUM") as psp:
            wq_sb = wp.tile([128, HT, 2, QD], F8)
            wk_sb = wp.tile([128, HT, 2, QD], F8)
            wv_sb = wp.tile([128, HT, 2, QD], F8)
            wo_sb = wp.tile([128, 2, H], F16)
            bk_sb = wp.tile([128, KT], F32)
            qT_sb = wp.tile([128, 2, S], F16)       # Q^T (q-dims on partitions)
            kT_sb = wp.tile([128, 2, KP], F16)      # K^T over packed keys
            va_sb = wp.tile([128, KT, NHL, HD + 1], F16)  # V rows + ones col
            a_sb = wp.tile([128, S // 128, QD], F16)      # A natural [q, a]
            aT_sb = wp.tile([128, 2, S], F16)             # A^T [a, q]
            xq_sb = wp.tile([128, NQB, HT, 2, QBW], F8)
            xp_sb = wp.tile([128, KT, HT, 2, 128], F8)

            # input DMAs (SP queue; order = fill-critical first: the first
            # exp needs xq0-3+wq (Q half0) and wk+xp0 (K kt0) only)
            id_sb = wp.tile([128, 128], F16)
            nc.sync.dma_start(out=bk_sb, in_=bk.ap())
            nc.sync.dma_start(out=wq_sb, in_=wqx.ap())
            for qb in range(4):
                nc.sync.dma_start(out=xq_sb[:, qb], in_=xq.ap()[:, qb])
            nc.sync.dma_start(out=wk_sb, in_=wkx.ap())
            for kb in range(0, 4):
                nc.sync.dma_start(out=xp_sb[:, kb], in_=xp.ap()[:, kb])
            nc.sync.dma_start(out=wv_sb, in_=wvx.ap())
            for kb in range(4, KT):
                nc.sync.dma_start(out=xp_sb[:, kb], in_=xp.ap()[:, kb])
            for qb in range(4, NQB):
                nc.sync.dma_start(out=xq_sb[:, qb], in_=xq.ap()[:, qb])
            for mt in range(2):
                nc.sync.dma_start(out=wo_sb[:, mt, :],
                                  in_=woT.ap()[mt * 128:(mt + 1) * 128, :])
            nc.sync.dma_start(out=id_sb, in_=ident.ap())
            nc.vector.memset(va_sb[:, :, :, HD:HD + 1], 1.0)
            # outT viewed as [128, 8, 2048]: row j = jt*128 + p
            outT_r = outT.ap().rearrange("(jt p) s -> p jt s", p=128)

            # ---------------- emitters ----------------
            def hilo(ps, lhs_main, rhs_main, lhs_cross, rhs_cross):
                """12 DR matmuls: 4 hi*hi pair-steps + 8 hi/lo cross steps."""
                for t in range(NTP):
                    nc.tensor.matmul(ps, lhs_main(t), rhs_main(t),
                                     start=(t == 0), stop=False,
                                     perf_mode=DRM)
                for ht in range(HT):
                    nc.tensor.matmul(ps, lhs_cross(ht), rhs_cross(ht),
                                     start=False, stop=(ht == HT - 1),
                                     perf_mode=DRM)

            def gen_ps():
                return psp.tile([128, 512], F32, tag="gen", name="ps_g")

            def kproj(cb, kb):
                ps = gen_ps()[:, 0:128]
                cs = slice(cb * 128, (cb + 1) * 128)
                hilo(ps,
                     lambda t: wk_sb[:, 2 * t:2 * t + 2, 1, cs],
                     lambda t: xp_sb[:, kb, 2 * t:2 * t + 2, 0, :],
                     lambda ht: wk_sb[:, ht, :, cs],
                     lambda ht: xp_sb[:, kb, ht, :, :])
                nc.vector.tensor_copy(kT_sb[:, cb, kb * 128:(kb + 1) * 128], ps)

            def vproj(kb):
                ps = gen_ps()[:, 0:QD]
                hilo(ps,
                     lambda t: xp_sb[:, kb, 2 * t:2 * t + 2, 0, :],
                     lambda t: wv_sb[:, 2 * t:2 * t + 2, 1, :],
                     lambda ht: xp_sb[:, kb, ht, :, :],
                     lambda ht: wv_sb[:, ht, :, :])
                # V carries a WS factor; remove it at evac time.
                nc.vector.tensor_scalar_mul(
                    va_sb[:, kb, :, 0:HD],
                    ps.rearrange("p (h d) -> p h d", h=NHL), 1.0 / WS)

            def qproj(cb, qb):
                ps = gen_ps()[:, 0:QBW]
                cs = slice(cb * 128, (cb + 1) * 128)
                hilo(ps,
                     lambda t: wq_sb[:, 2 * t:2 * t + 2, 1, cs],
                     lambda t: xq_sb[:, qb, 2 * t:2 * t + 2, 0, :],
                     lambda ht: wq_sb[:, ht, :, cs],
                     lambda ht: xq_sb[:, qb, ht, :, :])
                nc.vector.tensor_copy(qT_sb[:, cb, qb * QBW:(qb + 1) * QBW], ps)

            pex_tiles = {}

            def new_pex(h, half):
                pex_tiles[(h, half)] = pexp.tile(
                    [128, KT, 1024], F16, tag="pex", name=f"pex{h}_{half}")

            def scores_kt(h, kt, half, split=False):
                pex_t = pex_tiles[(h, half)]
                mtq, rb = h // 2, 64 * (h % 2)
                ps = scp.tile([128, 1024], F32, tag="sc", name="ps_s")
                q0 = half * 1024
                for co in (0, 512):
                    nc.tensor.matmul(
                        ps[:, co:co + 512],
                        kT_sb[rb:rb + 64, mtq, kt * 128:(kt + 1) * 128],
                        qT_sb[rb:rb + 64, mtq, q0 + co:q0 + co + 512],
                        start=True, stop=True)
                    if split:
                        # finer first exp: starts after only half the Q DMAs
                        nc.scalar.activation(
                            out=pex_t[:, kt, co:co + 512], in_=ps[:, co:co + 512],
                            func=EXP, bias=bk_sb[:, kt:kt + 1],
                            scale=SCALE / (WS * WS))
                if not split:
                    nc.scalar.activation(
                        out=pex_t[:, kt, :], in_=ps,
                        func=EXP, bias=bk_sb[:, kt:kt + 1],
                        scale=SCALE / (WS * WS))

            av_state = {}

            def av_qt(h, qt):
                if not do_av:
                    return
                pex_t = pex_tiles[(h, qt // 8)]
                j = qt % 4
                if j == 0:
                    av_state[h] = avp.tile([128, 4, HD + 1], F32, tag="av",
                                           name="pa")
                pa = av_state[h]
                qloc = (qt % 8) * 128
                for kt in range(KT):
                    nc.tensor.matmul(
                        pa[:, j, :], pex_t[:, kt, qloc:qloc + 128],
                        va_sb[:, kt, h, :],
                        start=(kt == 0), stop=(kt == KT - 1))
                rcp = wp.tile([128, 1], F32, tag="rcp", bufs=4, name="rcp")
                nc.vector.reciprocal(rcp, pa[:, j, HD:HD + 1])
                nc.vector.tensor_scalar_mul(
                    a_sb[:, qt, h * HD:(h + 1) * HD], pa[:, j, 0:HD], rcp)

            def tr_qt(at, qt, eng=None):
                # SBUF->SBUF XBAR transpose of one [128,128] fp16 block
                if not do_tr:
                    return
                (eng or nc.sync).dma_start(
                    out=aT_sb[:, at, qt * 128:(qt + 1) * 128],
                    in_=a_sb[:, qt, at * 128:(at + 1) * 128], transpose=True)

            stg_state = {}

            def op_jt(qg, jt, tail=False):
                """One [128j x 512q] out-proj chunk; the whole qg stages into
                one [128, 8, 512] SBUF tile and ships as a single DMA."""
                if not do_op:
                    return
                if jt == 0:
                    stg_state[qg] = wp.tile([128, HT, 512], F16, tag="stgg",
                                            bufs=2, name="stgg")
                stg = stg_state[qg]
                if tail and jt % 3 == 1:
                    # deep tail psum rotation: scores + AV pools are idle
                    # once the exp stream has drained.
                    po = scp.tile([128, 512], F32, tag="sc", name="po_sc")
                elif tail and jt % 3 == 2:
                    po = avp.tile([128, 512], F32, tag="av", name="po_av")
                else:
                    po = gen_ps()
                # mt=1 (the later-arriving A^T half) FIRST: its readiness
                # gates the chunk, so the scheduler cannot hoist the chunk
                # ahead of the exp stream and block the PE queue on it.
                for mt in (1, 0):
                    nc.tensor.matmul(
                        po, wo_sb[:, mt, jt * 128:(jt + 1) * 128],
                        aT_sb[:, mt, qg * 512:(qg + 1) * 512],
                        start=(mt == 1), stop=(mt == 0))
                if tail and jt % 2 == 1:
                    nc.scalar.copy(stg[:, jt, :], po)   # ACT is idle post-exp
                else:
                    nc.vector.tensor_copy(stg[:, jt, :], po)
                qs = slice(qg * 512, (qg + 1) * 512)
                if tail and qg == 3 and jt == 3:
                    # ship the first half early so the final transfer is short
                    nc.sync.dma_start(out=outT_r[:, 0:4, qs], in_=stg[:, 0:4, :])
                elif jt == HT - 1:
                    if tail and qg == 3:
                        nc.scalar.dma_start(out=outT_r[:, 4:8, qs],
                                            in_=stg[:, 4:8, :])
                    else:
                        eng = nc.scalar if tail else nc.sync
                        eng.dma_start(out=outT_r[:, :, qs], in_=stg)

            # ---------------- emission schedule ----------------
            # fill: minimal work before the first exp (Q first: its DMAs
            # land first; K's inputs arrive while Q projects)
            qproj(0, 0)
            qproj(0, 1)
            kproj(0, 0)
            qproj(0, 2)
            qproj(0, 3)

            def run_stream(h, half, side, split_kt0=False):
                new_pex(h, half)
                for kt in range(KT):
                    for fn in side.get(kt, ()):
                        fn()
                    scores_kt(h, kt, half, split=(split_kt0 and kt == 0))

            # pos0: h0 half0.  K(0,kt) feeds each kt; V waits for wv
            # (which lands after xp0-3) and does not gate any exp.
            side = {kt: [lambda k=kt: kproj(0, k)] for kt in range(1, KT)}
            for kt in range(5, KT):
                side[kt].extend([lambda k=2 * kt - 10: vproj(k),
                                 lambda k=2 * kt - 9: vproj(k)])
            run_stream(0, 0, side, split_kt0=True)
            # pos1: h1 half0.  K(1,*) and Q(1, qb0-3) (feeds h2/h3 half0).
            side = {0: [lambda: vproj(8), lambda: kproj(1, 0),
                        lambda: kproj(1, 1)]}
            for kt in range(1, 5):
                side[kt] = [lambda k=2 * kt: kproj(1, k)]
                if kt < 4:
                    side[kt].append(lambda k=2 * kt + 1: kproj(1, k))
            for kt in range(5, KT):
                side[kt] = [lambda q=kt - 5: qproj(1, q)]
            run_stream(1, 0, side)
            # pos2: h2 half0.  AV(h0, qt0-7) packed early, Q(0, qb4-7).
            side = {kt: [lambda q=2 * kt - 2: av_qt(0, q),
                         lambda q=2 * kt - 1: av_qt(0, q)]
                    for kt in range(1, 5)}
            for kt in range(5, KT):
                side[kt] = [lambda q=kt - 1: qproj(0, q)]
            run_stream(2, 0, side)
            # pos3: h3 half0.  AV(h1) then AV(h2); T(at0, qt0-7) late.
            side = {kt: [lambda q=2 * kt - 2: av_qt(1, q),
                         lambda q=2 * kt - 1: av_qt(1, q)]
                    for kt in range(1, 5)}
            for kt in range(5, KT):
                side[kt] = [lambda q=2 * kt - 10: av_qt(2, q),
                            lambda q=2 * kt - 9: av_qt(2, q)]
            side[6].extend([lambda q=q: tr_qt(0, q) for q in range(4)])
            side[KT - 1].extend([lambda q=q: tr_qt(0, q) for q in range(4, 8)])
            run_stream(3, 0, side)
            # pos4: h0 half1.  AV(h3, qt0-7), Q(1, qb4-7), T(at1, qt0-7).
            side = {kt: [lambda q=2 * kt - 2: av_qt(3, q),
                         lambda q=2 * kt - 1: av_qt(3, q)]
                    for kt in range(1, 5)}
            for kt in range(5, KT):
                side[kt] = [lambda q=kt - 5: qproj(1, 4 + q)]
            side[6].extend([lambda q=q: tr_qt(1, q) for q in range(4)])
            side[KT - 1].extend([lambda q=q: tr_qt(1, q) for q in range(4, 8)])
            run_stream(0, 1, side)
            # pos5-7: half1 streams; AV + OP(qg0/qg1) spread thinly.
            opq = [(qg, jt) for qg in (0, 1) for jt in range(HT)]
            side = {kt: [lambda q=7 + kt: av_qt(0, q)] for kt in range(1, KT)}
            for i, kt in enumerate((2, 3, 5, 6, 8)):
                side[kt].append(lambda c=opq[i]: op_jt(*c))
            run_stream(1, 1, side)
            side = {kt: [lambda q=7 + kt: av_qt(1, q)] for kt in range(1, KT)}
            for i, kt in enumerate((1, 2, 3, 5, 6, 8)):
                side[kt].append(lambda c=opq[5 + i]: op_jt(*c))
            side[6].extend([lambda q=q: tr_qt(0, q) for q in range(8, 12)])
            run_stream(2, 1, side)
            side = {kt: [lambda q=7 + kt: av_qt(2, q)] for kt in range(1, KT)}
            for i, kt in enumerate((1, 2, 4, 5, 6)):
                side[kt].append(lambda c=opq[11 + i]: op_jt(*c))
            side[3].extend([lambda q=q: tr_qt(0, q) for q in range(12, 16)])
            run_stream(3, 1, side)
            # tail: AV(h3, qt8-15) first (PE free-runs), transposes chase the
            # norms, then the two remaining out-projection query groups with
            # a 4-deep psum rotation (borrowing the idle scores-pool banks).
            for qt in range(8, 16):
                av_qt(3, qt)
            for qt in range(8, 16):
                tr_qt(1, qt, eng=nc.sync if qt % 2 == 0 else nc.scalar)
            for jt in range(HT):
                op_jt(2, jt, tail=True)
            for jt in range(HT):
                op_jt(3, jt, tail=True)

    nc.compile()
    return nc


def _hilo8(a):
    hi = np.asarray(a, np.float32).astype(E4)
    lo = (np.asarray(a, np.float32) - hi.astype(np.float32)).astype(E4)
    return hi, lo


def _pack_x(xT, blk):
    """[1024, N] -> [128, N//blk, 8, 2, blk] with (hi, lo) on axis 3."""
    n = xT.shape[1]
    hi, lo = _hilo8(xT)
    out = np.stack([hi.reshape(HT, 128, n // blk, blk),
                    lo.reshape(HT, 128, n // blk, blk)], axis=3)
    return np.ascontiguousarray(out.transpose(1, 2, 0, 3, 4))


def _pack_w(wT):
    """[1024, QD] -> [128, 8, 2, QD] with (lo, hi) on axis 2."""
    hi, lo = _hilo8(wT * WS)
    out = np.stack([lo.reshape(HT, 128, QD), hi.reshape(HT, 128, QD)], axis=2)
    return np.ascontiguousarray(out.transpose(1, 0, 2, 3))


def _prep_inputs(hidden_states, attention_mask, w_qkv, w_out):
    """Shard + pack inputs for the 8 cores.  Returns (KP, in_maps)."""
    hs = np.asarray(hidden_states, dtype=np.float32)
    mask = np.asarray(attention_mask)
    wqkv = np.asarray(w_qkv, dtype=np.float32)
    wo = np.asarray(w_out, dtype=np.float32)

    idxs = [np.nonzero(mask[b] != 0)[0] for b in range(B)]
    counts = [len(ix) for ix in idxs]
    KP = max(128, ((max(counts) + 127) // 128) * 128)
    KT = KP // 128

    xqs, xps, bks = [], [], []
    for b in range(B):
        xqs.append(_pack_x(np.ascontiguousarray(hs[b].T), QBW))
        xpad = np.zeros((KP, H), dtype=np.float32)
        xpad[:counts[b]] = hs[b][idxs[b]]
        xps.append(_pack_x(np.ascontiguousarray(xpad.T), 128))
        bias = np.zeros((KT, 128), dtype=np.float32)
        bias.reshape(-1)[counts[b]:] = -30000.0
        bks.append(np.ascontiguousarray(bias.T))

    in_maps = []
    for c in range(NCORES):
        b, g = c // CPB, c % CPB
        sl = slice(g * QD, (g + 1) * QD)
        in_maps.append({
            "xq": xqs[b],
            "xp": xps[b],
            "wqx": _pack_w(np.ascontiguousarray(wqkv[sl, :].T)),
            "wkx": _pack_w(np.ascontiguousarray(
                wqkv[H + sl.start:H + sl.stop, :].T)),
            "wvx": _pack_w(np.ascontiguousarray(
                wqkv[2 * H + sl.start:2 * H + sl.stop, :].T)),
            "woT": np.ascontiguousarray(wo[:, sl].T.astype(np.float16)),
            "bk": bks[b],
            "ident": np.eye(128, dtype=np.float16),
        })
    return KP, in_maps


_NC_CACHE = {}


def kernel(hidden_states, attention_mask, w_qkv, w_out):
    KP, in_maps = _prep_inputs(hidden_states, attention_mask, w_qkv, w_out)
    if KP not in _NC_CACHE:
        _NC_CACHE[KP] = build_kernel(KP)
    nc = _NC_CACHE[KP]
    res = bass_utils.run_bass_kernel_spmd(nc, in_maps,
                                          core_ids=list(range(NCORES)))
    out = np.empty((B, S, H), dtype=np.float32)
    for b in range(B):
        acc = res.results[b * CPB]["outT"].astype(np.float32)
        for c in range(b * CPB + 1, (b + 1) * CPB):
            acc = acc + res.results[c]["outT"].astype(np.float32)
        out[b] = acc.T
    return out
